# revision 32
# baseline (speedup 1.0000x reference)
"""Dual-path self-attention (DPSA) kernel for 8 Trainium2 NeuronCores.

Reference computation (B=2, S=2048, D=1024, H=16, DK=64):
    Q_sem = X_sem @ Wq_sem + bq_sem   (per-head)
    K_sem = X_sem @ Wk_sem + bk_sem
    V     = X_sem @ Wv + bv
    Q_sal = X_sal @ Wq_sal + bq_sal
    K_sal = X_sal @ Wk_sal + bk_sal
    A = (Q_sem K_sem^T + gamma * Q_sal K_sal^T) / sqrt(DK)
    A = softmax(mask ? A : -1e9)
    out = (A V) @ Wo + bo

Sharding: batch x head-group. Core c handles batch c//4 and heads
[4*(c%4), 4*(c%4)+4). Each core computes its 4 heads' partial output
projection sum_h(O_h @ Wo[rows_h]); the host reduces the 4 partials per
batch and adds bo.

Device-side layout ("transposed attention"):
  - X^T [D, S] resident in SBUF (fp16); projections produce Q^T/K^T
    directly: psum[m=channels, n=seq] = Wchunk.T @ X^T chunk.
  - QcatT/KcatT per head: [128, S] with semantic rows 0:64, salience
    rows 64:128. One contract-128 matmul computes
    A^T = (Q_sem K_sem^T + gamma Q_sal K_sal^T)^T / sqrt(DK) fused at
    full PE efficiency (scales folded into Q-side weights on host).
  - exp on ScalarE (PSUM->SBUF, fp16 out) for 6/8 key-tile groups; the
    other 2/8 use a DVE Schraudolph bit-trick exp (fp32 affine -> int16
    -> reinterpret fp16) to keep ScalarE off the critical path.
  - AV accumulation with lhsT = [V_h | ones] so the softmax denominator
    falls out as row 64 of the AV psum for free. AV matmuls trail their
    exp by one kt-pair so the PE never sits in an exp semaphore wait.
  - Normalization delayed past AV: O_unnorm^T scaled by 1/denom
    (DVE reciprocal straight from PSUM + fp16 pair-packed broadcast
    matmul) before Wo.
  - Startup: PE warmup matmuls (HAM stays at full clock) while X/W DMAs
    stream in, striped across the sync/scalar/gpsimd queues; the
    salience projections consume X^T contract-chunks as they land.
"""

import numpy as np

B, S, D, H = 2, 2048, 1024, 16
DK = D // H  # 64
N_CORES = 8
HG = 4  # head groups (cores per batch)
HPC = 4  # heads per core
DKC = HPC * DK  # 256 channels per core
QCHUNK = 512
NQC = S // QCHUNK  # 4
NKT = S // 128  # 16 key tiles
VSLOT = DK + 1  # V columns per (ktile, head) slot incl. ones column

# DVE Schraudolph exp: exp(x) ~= bitcast_fp16(int16(round(M*x + B)))
EXP_M = 1477.3152  # 2^10 / ln(2)
EXP_B = 15301.3  # 15*1024 - 58.7 (zero-mean log error)
N_WARMUP_MM = 26


def _exp_on_dve(gi, h, hp):
    # Route 1/4 of exp tiles (kt-pair groups 2 and 5, both streams) to
    # the DVE. Measured output max-rel-err 1.07e-2 for this assignment
    # (vs 1.81e-2 for a scattered per-stream split -- the max-err tail
    # is sensitive to placement), comfortably under the 2e-2 gate.
    return gi in (2, 5)

_cached = {}


def _build_nc(with_qk_bias, with_v_bias):
    import concourse.bass as bass
    import concourse.tile as tile
    from concourse import bacc, mybir

    fp16 = mybir.dt.float16
    fp32 = mybir.dt.float32
    i16 = mybir.dt.int16

    nc = bacc.Bacc(None)

    # ---- DRAM I/O (per-core shards) ----
    xt_sem_d = nc.dram_tensor("xt_sem", [D, S], fp16, kind="ExternalInput")
    xt_sal_d = nc.dram_tensor("xt_sal", [D, S], fp16, kind="ExternalInput")
    # weights pre-rearranged on host: w_r[p, c*C + j] = W[c*128 + p, j]
    wq_d = nc.dram_tensor("wq", [128, 8 * DKC], fp16, kind="ExternalInput")
    wk_d = nc.dram_tensor("wk", [128, 8 * DKC], fp16, kind="ExternalInput")
    wqs_d = nc.dram_tensor("wqs", [128, 8 * DKC], fp16, kind="ExternalInput")
    wks_d = nc.dram_tensor("wks", [128, 8 * DKC], fp16, kind="ExternalInput")
    wv_d = nc.dram_tensor("wv", [128, 8 * DKC], fp16, kind="ExternalInput")
    wo_d = nc.dram_tensor("wo", [128, 2 * D], fp16, kind="ExternalInput")
    if with_qk_bias:
        bqk_d = nc.dram_tensor("bqk", [1, 4 * DKC], fp16, kind="ExternalInput")
    if with_v_bias:
        bv_d = nc.dram_tensor("bv", [1, DKC], fp16, kind="ExternalInput")
    out_d = nc.dram_tensor("out", [S, D], fp16, kind="ExternalOutput")



    with tile.TileContext(nc) as tc:
        with (
            tc.tile_pool(name="persist", bufs=1) as persist,
            tc.tile_pool(name="psum", bufs=1, space="PSUM") as psum,
            tc.tile_pool(name="expp", bufs=7) as expp,
            tc.tile_pool(name="outp", bufs=3) as outp,
            tc.tile_pool(name="rp", bufs=4) as rp,
        ):
            qcat = persist.tile([128, HPC, S], fp16)
            kcat = persist.tile([128, HPC, S], fp16)
            v_sb = persist.tile([128, NKT, HPC, VSLOT], fp16)
            ones_sb = persist.tile([1, QCHUNK], fp16)
            warm_w = persist.tile([128, 128], fp16)
            warm_x = persist.tile([128, QCHUNK], fp16)
            wq_sb = persist.tile([128, 8 * DKC], fp16)
            wk_sb = persist.tile([128, 8 * DKC], fp16)
            wqs_sb = persist.tile([128, 8 * DKC], fp16)
            wks_sb = persist.tile([128, 8 * DKC], fp16)
            wv_sb = persist.tile([128, 8 * DKC], fp16)
            wo_sb = persist.tile([128, 2 * D], fp16)
            xt_sal = persist.tile([128, 8, S], fp16)
            xt_sem = persist.tile([128, 8, S], fp16)
            o_un = persist.tile([64, HPC, S], fp16)
            ot = persist.tile([128, 2, S], fp16)

            nc.vector.memset(ones_sb[:], 1.0)
            nc.vector.memset(warm_w[:], 0.5)
            nc.vector.memset(warm_x[:], 0.5)
            nc.vector.memset(v_sb[:, :, :, DK : DK + 1], 1.0)

            # ---- input DMAs: stripe across sync/scalar/gpsimd queues in
            # consumption order (weights for the first matmuls + X_sal
            # chunks first; X_sem, remaining weights, Wo behind them).
            xt_sal_r = xt_sal_d.rearrange("(c p) s -> c p s", p=128)
            xt_sem_r = xt_sem_d.rearrange("(c p) s -> c p s", p=128)
            nc.scalar.dma_start(out=wqs_sb[:], in_=wqs_d[:])
            nc.gpsimd.dma_start(out=wks_sb[:], in_=wks_d[:])
            _engs = {"sy": nc.sync, "sc": nc.scalar, "gp": nc.gpsimd}
            for kc, e in enumerate(["sy", "sc", "gp", "sy", "sc", "gp", "sy", "sc"]):
                _engs[e].dma_start(out=xt_sal[:, kc, :], in_=xt_sal_r[kc])
            nc.gpsimd.dma_start(out=wv_sb[:], in_=wv_d[:])
            nc.scalar.dma_start(out=wq_sb[:], in_=wq_d[:])
            nc.sync.dma_start(out=wk_sb[:], in_=wk_d[:])
            for kc, e in enumerate(["sc", "gp", "sy", "sc", "gp", "sy", "sc", "gp"]):
                _engs[e].dma_start(out=xt_sem[:, kc, :], in_=xt_sem_r[kc])
            nc.sync.dma_start(out=wo_sb[:], in_=wo_d[:])
            if with_qk_bias:
                bqk_sb = persist.tile([1, 4 * DKC], fp16)
                nc.sync.dma_start(out=bqk_sb[:], in_=bqk_d[:])
            if with_v_bias:
                bv_sb = persist.tile([1, DKC], fp16)
                nc.sync.dma_start(out=bv_sb[:], in_=bv_d[:])

            # ---- PE warmup: unthrottle HAM while the DMAs land ----
            for _ in range(N_WARMUP_MM):
                wps = psum.tile([128, QCHUNK], fp32, tag="wo", bufs=2, name="warm")
                nc.tensor.matmul(wps, warm_w[:], warm_x[:], start=True, stop=True)

            # ============ streaming projection pair (Q+K of one path) ======
            # kc-outer so each X^T contract-chunk is consumed as its DMA
            # lands and each weight chunk is loaded to the PE once per
            # 4 matmuls. Q strips live in the "sp" psum ring (2x 1024-col),
            # K strips in the "av"/"wo" rings (4x 512-col).
            def _emit_proj_pair(w1_sb, w2_sb, x_sb, row0, b1, b2, mt):
                qs = [
                    psum.tile([128, 2 * QCHUNK], fp32, tag="sp", bufs=2, name="pjq")
                    for _ in range(2)
                ]
                ks = [
                    psum.tile([128, QCHUNK], fp32, tag=("av" if i < 2 else "wo"),
                              bufs=2, name="pjk")
                    for i in range(4)
                ]
                for kc in range(8):
                    w1c = w1_sb[:, kc * DKC + mt * 128 : kc * DKC + (mt + 1) * 128]
                    w2c = w2_sb[:, kc * DKC + mt * 128 : kc * DKC + (mt + 1) * 128]
                    xc = x_sb[:, kc, :]
                    # K first: at a sweep boundary the K strips' (smaller)
                    # drains finish ~1us before the Q strips', so the next
                    # sweep restarts on the PE sooner.
                    for j in range(4):
                        nc.tensor.matmul(
                            ks[j],
                            w2c,
                            xc[:, j * QCHUNK : (j + 1) * QCHUNK],
                            start=(kc == 0),
                            stop=(kc == 7 and not with_qk_bias),
                        )
                    for j in range(4):
                        nc.tensor.matmul(
                            qs[j // 2][:, (j % 2) * QCHUNK : (j % 2 + 1) * QCHUNK],
                            w1c,
                            xc[:, j * QCHUNK : (j + 1) * QCHUNK],
                            start=(kc == 0),
                            stop=(kc == 7 and not with_qk_bias),
                        )
                if with_qk_bias:
                    bslice1 = bqk_sb[:, b1 * DKC + mt * 128 : b1 * DKC + (mt + 1) * 128]
                    bslice2 = bqk_sb[:, b2 * DKC + mt * 128 : b2 * DKC + (mt + 1) * 128]
                    for j in range(2):
                        for half in range(2):
                            nc.tensor.matmul(
                                qs[j][:, half * QCHUNK : (half + 1) * QCHUNK],
                                bslice1, ones_sb[:, :QCHUNK],
                                start=False, stop=True,
                            )
                    for j in range(4):
                        nc.tensor.matmul(
                            ks[j], bslice2, ones_sb[:, :QCHUNK],
                            start=False, stop=True,
                        )
                # drain to qcat/kcat (heads 2*mt, 2*mt+1)
                for j in range(2):
                    g_sl = slice(j * 2 * QCHUNK, (j + 1) * 2 * QCHUNK)
                    nc.vector.tensor_copy(qcat[row0 : row0 + 64, 2 * mt, g_sl],
                                          qs[j][0:64, :])
                    nc.scalar.copy(qcat[row0 : row0 + 64, 2 * mt + 1, g_sl],
                                   qs[j][64:128, :])
                for j in range(4):
                    g_sl = slice(j * QCHUNK, (j + 1) * QCHUNK)
                    nc.vector.tensor_copy(kcat[row0 : row0 + 64, 2 * mt, g_sl],
                                          ks[j][0:64, :])
                    nc.scalar.copy(kcat[row0 : row0 + 64, 2 * mt + 1, g_sl],
                                   ks[j][64:128, :])

            _emit_proj_pair(wqs_sb, wks_sb, xt_sal, 64, 1, 3, mt=0)
            _emit_proj_pair(wqs_sb, wks_sb, xt_sal, 64, 1, 3, mt=1)
            _emit_proj_pair(wq_sb, wk_sb, xt_sem, 0, 0, 2, mt=0)
            # sem mt=1 deferred into the attention stream (PE filler)

            # V: natural layout [s, dv]; two s-tiles per psum group
            def _emit_v(st2s):
                for st2 in st2s:
                    ps = psum.tile([128, 2 * QCHUNK], fp32, tag="sp", bufs=2, name="vps")
                    for j in range(2):
                        st = st2 * 2 + j
                        vp = ps[:, j * DKC : (j + 1) * DKC]
                        for kc in range(8):
                            nc.tensor.matmul(
                                vp,
                                xt_sem[:, kc, st * 128 : (st + 1) * 128],
                                wv_sb[:, kc * DKC : (kc + 1) * DKC],
                                start=(kc == 0),
                                stop=(kc == 7 and not with_v_bias),
                            )
                        if with_v_bias:
                            nc.tensor.matmul(
                                vp, ones_sb[:, :128], bv_sb[:], start=False, stop=True
                            )
                    nc.vector.tensor_copy(
                        v_sb[:, st2 * 2 : st2 * 2 + 2, :, 0:DK],
                        ps[:, : 2 * DKC].rearrange("p (t h d) -> p t h d", t=2, h=HPC),
                    )

            _emit_v([0, 1, 2, 3])

            # Wo for one qc's four 128-row output tiles (cc-outer so each
            # ot tile is loaded to the PE once for both 512-col halves)
            _ob_eng = [nc.sync, nc.gpsimd]

            def _emit_wo(sts, vec_only=False):
                for st in sts:
                    ob = outp.tile([128, D], fp16)
                    wps = [
                        psum.tile([128, 512], fp32, tag="wo", bufs=2, name=f"wop{nh}")
                        for nh in range(2)
                    ]
                    for cc in range(2):
                        for nh in range(2):
                            nc.tensor.matmul(
                                wps[nh],
                                ot[:, cc, st * 128 : (st + 1) * 128],
                                wo_sb[:, cc * D + nh * 512 : cc * D + (nh + 1) * 512],
                                start=(cc == 0),
                                stop=(cc == 1),
                            )
                    nc.vector.tensor_copy(ob[:, 0:512], wps[0][:])
                    if vec_only:
                        # mid-attention ScalarE has no slack; keep both
                        # drain halves on the DVE
                        nc.vector.tensor_copy(ob[:, 512:1024], wps[1][:])
                    else:
                        nc.scalar.copy(ob[:, 512:1024], wps[1][:])
                    _ob_eng[st % 2].dma_start(
                        out=out_d[st * 128 : (st + 1) * 128, :], in_=ob[:]
                    )

            # ===== attention, qc-outer; AV matmuls trail exp by one
            # ===== kt-pair so the PE never waits on ScalarE/DVE exp.
            wo_pending = []
            norm_pending = [None]
            for qc in range(NQC):
                q_sl = slice(qc * QCHUNK, (qc + 1) * QCHUNK)
                for hp in (0, 2):
                    avs = {}
                    for h in (hp, hp + 1):
                        avs[h] = psum.tile([65, QCHUNK], fp32, tag="av",
                                           name=f"av{h}", bufs=2)
                    first = {hp: True, hp + 1: True}
                    pend = []  # (h, et, kt0)
                    fill_wo = wo_pending[:2]
                    del wo_pending[:2]

                    def _flush_one():
                        h0, et0, kt0 = pend.pop(0)
                        for g in range(2):
                            nc.tensor.matmul(
                                avs[h0],
                                v_sb[:, kt0 + g, h0, :],
                                et0[:, g * QCHUNK : (g + 1) * QCHUNK],
                                start=(first[h0] and g == 0),
                                stop=(kt0 + g == NKT - 1),
                            )
                        first[h0] = False

                    for kt in range(0, NKT, 2):
                        gi = kt // 2
                        for h in (hp, hp + 1):
                            sp = psum.tile([128, 2 * QCHUNK], fp32, tag="sp", bufs=2)
                            for g in range(2):
                                nc.tensor.matmul(
                                    sp[:, g * QCHUNK : (g + 1) * QCHUNK],
                                    kcat[:, h, (kt + g) * 128 : (kt + g + 1) * 128],
                                    qcat[:, h, q_sl],
                                    start=True,
                                    stop=True,
                                )
                            et = expp.tile([128, 2 * QCHUNK], fp16)
                            if _exp_on_dve(gi, h, hp):
                                nc.vector.tensor_scalar(
                                    out=et[:].bitcast(i16),
                                    in0=sp[:],
                                    scalar1=EXP_M,
                                    scalar2=EXP_B,
                                    op0=mybir.AluOpType.mult,
                                    op1=mybir.AluOpType.add,
                                )
                            else:
                                nc.scalar.activation(
                                    et[:], sp[:], mybir.ActivationFunctionType.Exp
                                )
                            pend.append((h, et, kt))
                        while len(pend) > 4:
                            _flush_one()
                        # previous section's deferred normalize-B: by now
                        # its reciprocal chain on the DVE has finished, so
                        # the bc matmuls issue without stalling the PE
                        if gi == 0 and norm_pending[0] is not None:
                            norm_pending[0]()
                            norm_pending[0] = None
                        # fillers: remaining V tiles during the first stream,
                        # pending Wo drains afterwards
                        if qc == 0 and hp == 0 and gi < 4:
                            _emit_v([4 + gi])
                        if gi == 2 and len(fill_wo) > 0:
                            _emit_wo(fill_wo[:1], vec_only=True)
                        if gi == 5 and len(fill_wo) > 1:
                            _emit_wo(fill_wo[1:2], vec_only=True)
                    while pend:
                        _flush_one()

                    # normalize part A: drain O^T and 1/denom per head
                    rtmp16 = {}
                    for h in (hp, hp + 1):
                        nc.vector.tensor_copy(o_un[:, h, q_sl], avs[h][0:64, :])
                        dtmp = rp.tile([1, QCHUNK], fp32, tag="d32", bufs=4,
                                       name=f"d32_{h}")
                        nc.scalar.copy(dtmp, avs[h][64:65, :])
                        rtmp32 = rp.tile([1, QCHUNK], fp32, tag="r32", bufs=4,
                                         name=f"r32_{h}")
                        nc.vector.reciprocal_approx_fast(out=rtmp32, in_=dtmp)
                        rtmp16[h] = rp.tile([1, QCHUNK], fp16, tag="r16", bufs=4,
                                            name=f"r16_{h}")
                        nc.vector.tensor_copy(rtmp16[h], rtmp32)
                    # after qc0's first head pair: emit the deferred sem
                    # mt=1 projections (ready PE work behind the exp queue)
                    if qc == 0 and hp == 0:
                        _emit_proj_pair(wq_sb, wk_sb, xt_sem, 0, 0, 2, mt=1)
                    # normalize part B, deferred into the next section's
                    # kt loop: fp16 broadcast matmuls (shared ones lhsT;
                    # separate psum banks so the DVE read of one head's bc
                    # never collides with the other's PE write) + scale
                    def _norm_b(hp=hp, q_sl=q_sl, rtmp16=rtmp16):
                        bcs = {}
                        for h in (hp, hp + 1):
                            bcs[h] = psum.tile([64, QCHUNK], fp32, tag="wo",
                                               bufs=2, name=f"bc{h}")
                            nc.tensor.matmul(
                                bcs[h], ones_sb[0:1, 0:64], rtmp16[h][:],
                                start=True, stop=True,
                            )
                        for h in (hp, hp + 1):
                            nc.vector.tensor_tensor(
                                ot[(h % 2) * 64 : (h % 2) * 64 + 64, h // 2, q_sl],
                                o_un[:, h, q_sl],
                                bcs[h][:],
                                mybir.AluOpType.mult,
                            )

                    norm_pending[0] = _norm_b
                wo_pending.extend(range(qc * 4, qc * 4 + 4))

            norm_pending[0]()
            _emit_wo(wo_pending)

    nc.compile()
    return nc


def _get_nc(key):
    if key not in _cached:
        _cached[key] = _build_nc(*key)
    return _cached[key]


def _host_reference(X_sem, X_sal, mask, Wq_sem, bq_sem, Wk_sem, bk_sem, Wv,
                    bv, Wq_sal, bq_sal, Wk_sal, bk_sal, Wo, bo, gamma):
    f32 = np.float32
    scale = f32(1.0 / np.sqrt(DK))

    def heads(x):
        return x.reshape(B, S, H, DK).transpose(0, 2, 1, 3)

    def lin(x, W, b):
        return (x.reshape(B * S, D) @ np.asarray(W, f32)).reshape(B, S, D) + np.asarray(b, f32)

    Xm = np.asarray(X_sem, f32)
    Xl = np.asarray(X_sal, f32)
    Q = heads(lin(Xm, Wq_sem, bq_sem))
    K = heads(lin(Xm, Wk_sem, bk_sem))
    V = heads(lin(Xm, Wv, bv))
    Ql = heads(lin(Xl, Wq_sal, bq_sal))
    Kl = heads(lin(Xl, Wk_sal, bk_sal))
    out = np.empty((B, S, D), f32)
    for b in range(B):
        for h in range(H):
            A = (Q[b, h] @ K[b, h].T + gamma * (Ql[b, h] @ Kl[b, h].T)) * scale
            A = np.where(np.asarray(mask)[b, 0] == 0, f32(-1e9), A)
            A -= A.max(axis=-1, keepdims=True)
            np.exp(A, out=A)
            A /= A.sum(axis=-1, keepdims=True)
            out[b, :, h * DK : (h + 1) * DK] = A @ V[b, h]
    y = out.reshape(B * S, D) @ np.asarray(Wo, f32)
    return (y + np.asarray(bo, f32)).reshape(B, S, D)


def _rearrange_w(w):
    # [1024, C] -> [128, 8*C] with w_r[p, c*C + j] = w[c*128 + p, j]
    C = w.shape[1]
    return np.ascontiguousarray(
        w.reshape(8, 128, C).transpose(1, 0, 2).reshape(128, 8 * C)
    )


def _run_spmd_fast(nc, in_maps, n_cores):
    """run_bass_via_pjrt's multi-core path, but downloading each output
    array once instead of once per core (the stock helper re-gathers the
    sharded global for every core slice -- ~0.3s x 8 over the tunnel)."""
    import jax
    import numpy as _np
    from jax.sharding import Mesh, PartitionSpec
    from jax.experimental.shard_map import shard_map
    from concourse import mybir
    from concourse.bass2jax import (
        _bass_exec_p,
        install_neuronx_cc_hook,
        partition_id_tensor,
    )

    install_neuronx_cc_hook()
    partition_name = nc.partition_id_tensor.name if nc.partition_id_tensor else None
    in_names, out_names, out_avals, zero_outs = [], [], [], []
    for alloc in nc.m.functions[0].allocations:
        if not isinstance(alloc, mybir.MemoryLocationSet):
            continue
        name = alloc.memorylocations[0].name
        if alloc.kind == "ExternalInput":
            if name != partition_name:
                in_names.append(name)
        elif alloc.kind == "ExternalOutput":
            dt = mybir.dt.np(alloc.dtype)
            out_names.append(name)
            out_avals.append(jax.core.ShapedArray(tuple(alloc.tensor_shape), dt))
            zero_outs.append(_np.zeros(tuple(alloc.tensor_shape), dt))
    n_params, n_outs = len(in_names), len(out_names)
    in_names = in_names + out_names + ([partition_name] if partition_name else [])
    donate = tuple(range(n_params, n_params + n_outs))

    def _body(*args):
        operands = list(args)
        if partition_name is not None:
            operands.append(partition_id_tensor())
        return tuple(_bass_exec_p.bind(
            *operands,
            out_avals=tuple(out_avals),
            in_names=tuple(in_names),
            out_names=tuple(out_names),
            lowering_input_output_aliases=(),
            sim_require_finite=True,
            sim_require_nnan=True,
            nc=nc,
        ))

    devices = jax.devices()[:n_cores]
    mesh = Mesh(_np.asarray(devices), ("core",))
    sharded = jax.jit(
        shard_map(_body, mesh=mesh, in_specs=(PartitionSpec("core"),) * (n_params + n_outs),
                  out_specs=(PartitionSpec("core"),) * n_outs, check_rep=False),
        donate_argnums=donate, keep_unused=True,
    )
    concat_in = [
        _np.concatenate([m[in_names[i]] for m in in_maps], axis=0)
        for i in range(n_params)
    ]
    concat_zeros = [
        _np.zeros((n_cores * z.shape[0], *z.shape[1:]), z.dtype) for z in zero_outs
    ]
    out_arrs = sharded(*concat_in, *concat_zeros)
    gathered = [
        _np.asarray(a).reshape(n_cores, *out_avals[i].shape)
        for i, a in enumerate(out_arrs)
    ]
    return [
        {name: gathered[i][c] for i, name in enumerate(out_names)}
        for c in range(n_cores)
    ]


def kernel(X_sem, X_sal, mask, Wq_sem, bq_sem, Wk_sem, bk_sem, Wv, bv,
           Wq_sal, bq_sal, Wk_sal, bk_sal, Wo, bo, gamma):
    from concourse.bass_utils import run_bass_kernel_spmd

    X_sem = np.asarray(X_sem)
    X_sal = np.asarray(X_sal)
    mask = np.asarray(mask)
    f32 = np.float32
    scale = f32(1.0 / np.sqrt(DK))
    g = f32(np.asarray(gamma).reshape(()))

    wq_full = (np.asarray(Wq_sem) * scale).astype(np.float16)
    bq_full = (np.asarray(bq_sem) * scale).astype(np.float16)
    wqs_full = (np.asarray(Wq_sal) * (g * scale)).astype(np.float16)
    bqs_full = (np.asarray(bq_sal) * (g * scale)).astype(np.float16)
    wk_full = np.asarray(Wk_sem).astype(np.float16)
    bk_full = np.asarray(bk_sem).astype(np.float16)
    wks_full = np.asarray(Wk_sal).astype(np.float16)
    bks_full = np.asarray(bk_sal).astype(np.float16)
    wv_full = np.asarray(Wv).astype(np.float16)
    bv_full = np.asarray(bv).astype(np.float16)
    wo_full = np.asarray(Wo).astype(np.float16)

    if not bool(np.all(mask)):
        # Masks with zeros never occur in this problem's input spec
        # (fill: ones); handle them exactly via a host fallback.
        return _host_reference(
            X_sem, X_sal, mask, Wq_sem, bq_sem, Wk_sem, bk_sem, Wv, bv,
            Wq_sal, bq_sal, Wk_sal, bk_sal, Wo, bo, g,
        )

    with_qk_bias = bool(
        np.any(np.asarray(bq_sem)) or np.any(np.asarray(bq_sal))
        or np.any(np.asarray(bk_sem)) or np.any(np.asarray(bk_sal))
    )
    with_v_bias = bool(np.any(np.asarray(bv)))

    nc = _get_nc((with_qk_bias, with_v_bias))

    xt = []
    for b in range(B):
        xt.append((
            np.ascontiguousarray(X_sem[b].T.astype(np.float16)),
            np.ascontiguousarray(X_sal[b].T.astype(np.float16)),
        ))

    in_maps = []
    for c in range(N_CORES):
        b, hg = c // HG, c % HG
        blk = slice(hg * DKC, (hg + 1) * DKC)
        m = {
            "xt_sem": xt[b][0],
            "xt_sal": xt[b][1],
            "wq": _rearrange_w(wq_full[:, blk]),
            "wk": _rearrange_w(wk_full[:, blk]),
            "wqs": _rearrange_w(wqs_full[:, blk]),
            "wks": _rearrange_w(wks_full[:, blk]),
            "wv": _rearrange_w(wv_full[:, blk]),
            "wo": np.ascontiguousarray(
                wo_full[blk].reshape(2, 128, D).transpose(1, 0, 2).reshape(128, 2 * D)
            ),
        }
        if with_qk_bias:
            m["bqk"] = np.concatenate(
                [bq_full[blk], bqs_full[blk], bk_full[blk], bks_full[blk]]
            ).reshape(1, 4 * DKC)
        if with_v_bias:
            m["bv"] = bv_full[blk].reshape(1, DKC)
        in_maps.append(m)

    try:
        results = _run_spmd_fast(nc, in_maps, N_CORES)
    except Exception:
        results = run_bass_kernel_spmd(
            nc, in_maps, core_ids=list(range(N_CORES))
        ).results

    out = np.zeros((B, S, D), dtype=f32)
    for c in range(N_CORES):
        out[c // HG] += results[c]["out"].astype(f32)
    out += np.asarray(bo).astype(f32)
    return out
